# revision 24
# baseline (speedup 1.0000x reference)
"""Causal self-attention (B=4, T=2048, C=1024, H=16) on 8 trn2 NeuronCores.

Sharding: tensor-parallel over heads (2 heads/core) for QKV projection +
attention, then an on-device AllToAll reshards head-sharded -> row-sharded
so each core computes the output projection for its 1024 rows. Host gather
is pure concatenation.

All-bf16 datapath (fp8 DoubleRow measured at 1 cyc/col on this silicon —
no win). Structural changes over the original baseline:
- V is computed transposed (vT = wv^T x, stationary weights reused across
  the chunk) and moved into PV layout by DMA xbar transposes; removes the
  LDWEIGHTS-bound x-stationary V matmuls.
- Phase 1 and the qc{0,2} attention pass interleave per batch so the ACT
  engine starts consuming exps ~50us earlier.
- The gpsimd queue carries ONLY the initial memsets and the two AllToAll
  collectives: Tile's threshold waits on an engine's completion counter
  make anything that depends on a post-collective op on the same queue
  transitively wait out the whole collective (38us convoy in the
  baseline). The softmax-denominator broadcast therefore runs as a rank-1
  f32r matmul on PE instead of gpsimd partition_broadcast.
- Each chunk's normalization tail (broadcast, yT multiply, a2a write) is
  deferred into the next chunk so the PE queue never blocks on the DVE
  reciprocal latency.
- proj_half(0) is emitted after fire_a2a(1): its matmuls keep PE busy
  while the second collective flies; the a2a tail is pure PE work.

Layout trick (from the baseline): attention is computed transposed,
S^T[k, q] = K Q^T; V is augmented with a ones column so the PV matmul
accumulates the softmax denominator in row 64 of the same PSUM tile. No
max-subtraction (logits are small), no P transpose anywhere.
"""

import sys

for _p in ("/opt/trn_rl_repo",):
    if _p not in sys.path:
        sys.path.insert(0, _p)

import numpy as np
import ml_dtypes

B, T, C, H, HS = 4, 2048, 1024, 16, 64
NCORES = 8
HPC = H // NCORES            # heads per core = 2
CPC = HPC * HS               # channels per core = 128
ROWS = B * T                 # 8192
RPC = ROWS // NCORES         # rows per core = 1024
NKT = T // 128               # k-tiles per batch = 16

BF16 = ml_dtypes.bfloat16

_CACHE: dict = {}


def _apply_tile_tail_patch(tile_mod):
    """This container's walrus rejects CTRL-class instructions (Drain/NoOp)
    carrying semaphore waits. Re-emit TileContext's tail waits as individual
    EventSemaphore waits and use the sem-only barrier variant."""
    import bass_rust
    from concourse.vector_clock import ScopedClock

    if getattr(tile_mod.TileContext, "_tail_patch_applied", False):
        return

    def _drain_and_barrier(self, tick_clock, wait_clock):
        collector = self.nc.sync.nop(nofuse=True, hint="tile_tail_wait")
        wait_clock.add_sem_waits(
            collector.ins, ScopedClock({None: tick_clock.global_clock})
        )
        si = collector.ins.sync_info
        waits = list(si.on_wait) if si is not None else []
        collector.ins.sync_info = None
        for w in waits:
            assert w.wait_mode == "sem-ge-imm", w
            self.nc.sync.wait_ge(
                bass_rust.SemaphoreHandle(w.ant_name, w.id), w.wait_value
            )

        self.nc.all_engine_barrier(sem_only=True)
        assert self.sems is not None
        popped = self.nc._tile_sem_poison_stack.pop()
        assert popped is self._sem_poison
        self.nc.clear_and_free_semaphores(list(self.sems.allocated().values()))
        self.nc.all_engine_barrier(sem_only=True)

    tile_mod.TileContext._drain_and_barrier = _drain_and_barrier
    tile_mod.TileContext._tail_patch_applied = True


def _build():
    import concourse.bass as bass
    import concourse.bacc as bacc
    import concourse.mybir as mybir
    import concourse.tile as tile

    _apply_tile_tail_patch(tile)

    dt = mybir.dt
    F32 = dt.float32
    BF = dt.bfloat16
    Exp = mybir.ActivationFunctionType.Exp

    nc = bacc.Bacc(num_devices=NCORES)

    # Inputs (per-core unless noted). xT is x transposed: [C, B*T].
    xT = nc.dram_tensor("xT", [C, ROWS], BF, kind="ExternalInput")
    wqk = nc.dram_tensor("wqk", [C, 2 * CPC], BF, kind="ExternalInput")
    wv = nc.dram_tensor("wv", [C, CPC], BF, kind="ExternalInput")
    bq = nc.dram_tensor("bq", [CPC, 1], F32, kind="ExternalInput")   # prescaled 1/8
    bk = nc.dram_tensor("bk", [CPC, 1], F32, kind="ExternalInput")
    wp = nc.dram_tensor("wp", [C, C], BF, kind="ExternalInput")      # full c_proj_w
    bprime = nc.dram_tensor("bprime", [1, C], BF, kind="ExternalInput")
    maskd = nc.dram_tensor("maskd", [128, 128], BF, kind="ExternalInput")
    out = nc.dram_tensor("out", [RPC, C], F32, kind="ExternalOutput")

    with tile.TileContext(nc) as tc:
        with (
            tc.tile_pool(name="const", bufs=1) as constp,
            tc.tile_pool(name="big", bufs=1) as bigp,
            tc.tile_pool(name="xin", bufs=4) as xinp,
            tc.tile_pool(name="pt", bufs=4) as ptp,
            tc.tile_pool(name="work", bufs=2) as workp,
            tc.tile_pool(name="yt", bufs=4) as ytp,
            tc.tile_pool(name="st", bufs=3, space="PSUM") as stp,
            tc.tile_pool(name="ypsp", bufs=2, space="PSUM") as ypsp,
            tc.tile_pool(name="misc", bufs=3, space="PSUM") as miscp,
            tc.tile_pool(name="dram", bufs=1, space="DRAM") as dramp,
        ):
            # ---- constants ----
            wqk_sb = constp.tile([128, 8, 2 * CPC], BF, tag="wqk")
            nc.sync.dma_start(wqk_sb[:], wqk.rearrange("(ct p) o -> p ct o", p=128))
            wv_sb = constp.tile([128, 8, CPC], BF, tag="wv")
            nc.sync.dma_start(wv_sb[:], wv.rearrange("(ct p) o -> p ct o", p=128))
            wp_sb = constp.tile([128, 8, C], BF, tag="wp")
            nc.sync.dma_start(wp_sb[:], wp.rearrange("(ct p) o -> p ct o", p=128))
            bq_sb = constp.tile([CPC, 1], F32, tag="bq")
            nc.sync.dma_start(bq_sb[:], bq[:])
            bk_sb = constp.tile([CPC, 1], F32, tag="bk")
            nc.sync.dma_start(bk_sb[:], bk[:])
            bprime_sb = constp.tile([1, C], BF, tag="bprime")
            nc.sync.dma_start(bprime_sb[:], bprime[:])
            mask_sb = constp.tile([128, 128], BF, tag="mask")
            nc.sync.dma_start(mask_sb[:], maskd[:])
            ones_sb = constp.tile([1, 128], BF, tag="ones")
            nc.vector.memset(ones_sb[:], 1.0)
            ones_r = constp.tile([1, 64], dt.float32r, tag="onesr")
            nc.vector.tensor_copy(out=ones_r[:], in_=ones_sb[:1, 0:64])
            bq8_sb = constp.tile([CPC, 1], F32, tag="bq8")
            nc.vector.tensor_scalar(
                bq8_sb[:], bq_sb[:], 0.125, None, mybir.AluOpType.mult
            )
            Ident = mybir.ActivationFunctionType.Identity

            # ---- persistent intermediates ----
            # qT/kT: per-head slabs zero-padded d=64 -> 128 partitions so the
            # QK matmuls drive the full PE array (full-row contraction keeps
            # the HAM clock at 2.4 GHz).
            qT_sb = bigp.tile([128, HPC, ROWS], BF, tag="qT")
            kT_sb = bigp.tile([128, HPC, ROWS], BF, tag="kT")
            nc.gpsimd.memset(qT_sb[64:128, :, :], 0.0)
            nc.gpsimd.memset(kT_sb[64:128, :, :], 0.0)
            # v' per global k-tile: [128 rows, 64 slots, 2 heads * 128]; per
            # head slot: [64 v cols | ones col | 63 zero cols].
            vp_sb = bigp.tile([128, NKT * B, 2 * 128], BF, tag="vp")
            nc.gpsimd.memset(vp_sb[:, :, 65:128], 0.0)
            nc.gpsimd.memset(vp_sb[:, :, 193:256], 0.0)
            nc.vector.memset(vp_sb[:, :, 64:65], 1.0)
            nc.vector.memset(vp_sb[:, :, 192:193], 1.0)

            # AllToAll buffers: half A = dest cores' local rows 0:512
            # (qc 0,2), half B = rows 512:1024 (qc 1,3).
            a2a_in_h = [dramp.tile([NCORES * CPC, RPC // 2], BF, name=f"a2a_in{i}") for i in range(2)]
            a2a_out_h = [dramp.tile([NCORES * CPC, RPC // 2], BF, name=f"a2a_out{i}") for i in range(2)]

            xT_r = xT.rearrange("(ct p) r -> p ct r", p=128)

            def load_chunk(r):
                rs = slice(r * 512, (r + 1) * 512)
                xt = xinp.tile([128, 8, 512], BF, tag="xt", name=f"xt_{r}")
                nc.sync.dma_start(xt[:], xT_r[:, :, rs])
                return xt

            # ---------------- Phase 1: QKV projection ---------------------
            def p1_chunk(r, xt):
                rs = slice(r * 512, (r + 1) * 512)
                q_ps = miscp.tile([128, 512], F32, tag="ps", name=f"qps_{r}")
                k_ps = miscp.tile([128, 512], F32, tag="ps", name=f"kps_{r}")
                vt_ps = miscp.tile([128, 512], F32, tag="ps", name=f"vtps_{r}")
                for ct in range(8):
                    st, sp = (ct == 0), (ct == 7)
                    nc.tensor.matmul(
                        q_ps[:], wqk_sb[:, ct, 0:CPC], xt[:, ct, :], start=st, stop=sp
                    )
                    nc.tensor.matmul(
                        k_ps[:], wqk_sb[:, ct, CPC:], xt[:, ct, :], start=st, stop=sp
                    )
                    nc.tensor.matmul(
                        vt_ps[:], wv_sb[:, ct, :], xt[:, ct, :], start=st, stop=sp
                    )
                # copy-out with bias (per-partition) and 1/8 scale folded
                # into q (bq prescaled on host)
                for hh in range(HPC):
                    hs = slice(hh * 64, (hh + 1) * 64)
                    nc.scalar.activation(
                        qT_sb[0:64, hh, rs], q_ps[hs, :], Ident,
                        bias=bq8_sb[hs, :], scale=0.125,
                    )
                    nc.scalar.activation(
                        kT_sb[0:64, hh, rs], k_ps[hs, :], Ident,
                        bias=bk_sb[hs, :], scale=1.0,
                    )
                # vT -> bf16 staging -> xbar transpose into vp slots
                vt_sb = workp.tile([128, 512], BF, tag="vt", name=f"vt_{r}")
                nc.vector.tensor_copy(out=vt_sb[:], in_=vt_ps[:])
                for hh in range(HPC):
                    nc.sync.dma_start_transpose(
                        vp_sb[:, 4 * r : 4 * r + 4, hh * 128 : hh * 128 + 64],
                        vt_sb[hh * 64 : (hh + 1) * 64, :],
                    )

            # ---------------- Phase 2: attention --------------------------
            pending = []   # deferred normalization-tail closures

            def flush_pending():
                while pending:
                    pending.pop(0)()

            def attn_chunk(b, h, qc):
                vc = slice(h * 128, h * 128 + 128)
                q0 = qc * 512
                grow = b * T + q0
                dest = grow // RPC
                half = (grow % RPC) // 512
                y_ps = ypsp.tile([128, 512], F32, tag="yps", name=f"yps_{b}_{h}_{qc}")
                nkt = 4 * qc + 4

                def qk_exp(ki):
                    diag = ki // 4 == qc
                    n = 512 - (ki - 4 * qc) * 128 if diag else 512
                    qs0 = q0 + 512 - n
                    qsl = slice(b * T + qs0, b * T + q0 + 512)
                    st_ps = stp.tile(
                        [128, 512], F32, tag="st", name=f"st_{b}_{h}_{qc}_{ki}"
                    )
                    nc.tensor.matmul(
                        st_ps[:, :n],
                        kT_sb[:, h, b * T + ki * 128 : b * T + (ki + 1) * 128],
                        qT_sb[:, h, qsl],
                        start=True,
                        stop=True,
                    )
                    pT = ptp.tile([128, 512], BF, tag="pT")
                    nc.scalar.activation(pT[:, :n], st_ps[:, :n], Exp)
                    if diag:
                        nc.vector.tensor_tensor(
                            pT[:, 0:128], pT[:, 0:128], mask_sb[:],
                            mybir.AluOpType.mult,
                        )
                    return pT, n

                # software pipeline depth 3: QK/exp for ki+3 issues before
                # PV(ki); the deferred tail of the previous chunk flushes
                # after the first QK so PE never waits on its reciprocal.
                pend = [qk_exp(0)]
                for ki in range(1, min(3, nkt)):
                    pend.append(qk_exp(ki))
                flush_pending()
                for ki in range(nkt):
                    pT, n = pend.pop(0)
                    if ki + 3 < nkt:
                        pend.append(qk_exp(ki + 3))
                    nc.tensor.matmul(
                        y_ps[:, 512 - n :],
                        vp_sb[:, b * NKT + ki, vc],
                        pT[:, :n],
                        start=(ki == 0),
                        stop=(ki == nkt - 1),
                    )

                # normalization: reciprocal of the denominator row, then a
                # deferred tail (PE rank-1 broadcast + DVE multiply + a2a
                # write) emitted from the next chunk.
                den = workp.tile([1, 512], F32, tag="den")
                nc.vector.tensor_copy(out=den[:], in_=y_ps[64:65, :])
                rcp_f = workp.tile([1, 512], F32, tag="rcpf")
                nc.vector.reciprocal_approx_fast(rcp_f[:], den[:])
                rcp = workp.tile([1, 512], dt.float32r, tag="rcp")
                nc.vector.tensor_copy(out=rcp[:], in_=rcp_f[:])

                def tail(y_ps=y_ps, rcp=rcp, b=b, h=h, qc=qc, dest=dest, half=half):
                    bc_ps = miscp.tile(
                        [64, 512], F32, tag="ps", name=f"bcps_{b}_{h}_{qc}"
                    )
                    nc.tensor.matmul(
                        bc_ps[:], ones_r[:], rcp[:], start=True, stop=True
                    )
                    bc = workp.tile([64, 512], F32, tag="bc")
                    nc.vector.tensor_copy(out=bc[:], in_=bc_ps[:])
                    yT = ytp.tile([64, 512], BF, tag="yT", name=f"yT_{b}_{h}_{qc}")
                    nc.vector.tensor_tensor(
                        yT[:], y_ps[0:64, :], bc[:], mybir.AluOpType.mult
                    )
                    nc.sync.dma_start(
                        a2a_in_h[half][
                            dest * CPC + h * 64 : dest * CPC + (h + 1) * 64, :
                        ],
                        yT[:],
                    )

                pending.append(tail)

            def fire_a2a(half):
                nc.gpsimd.collective_compute(
                    "AllToAll",
                    mybir.AluOpType.bypass,
                    replica_groups=[list(range(NCORES))],
                    ins=[a2a_in_h[half][:].opt()],
                    outs=[a2a_out_h[half][:].opt()],
                )

            # ---------------- Phase 3: output projection ------------------
            _gathered = {}

            def proj_half(half, rts=(0, 1, 2, 3)):
                if half not in _gathered:
                    yTh = bigp.tile([128, 8, RPC // 2], BF, tag=f"yTall{half}")
                    nc.sync.dma_start(
                        yTh[:],
                        a2a_out_h[half][:].rearrange("(ct p) r -> p ct r", p=128),
                    )
                    _gathered[half] = yTh
                yTh = _gathered[half]
                out_r = out.rearrange("(rt p) o -> p rt o", p=128)
                for rt in rts:
                    for oc in range(2):
                        ocs = slice(oc * 512, (oc + 1) * 512)
                        o_ps = miscp.tile(
                            [128, 512], F32, tag="ps", name=f"ops_{half}_{rt}_{oc}"
                        )
                        for ct in range(8):
                            nc.tensor.matmul(
                                o_ps[:],
                                yTh[:, ct, rt * 128 : (rt + 1) * 128],
                                wp_sb[:, ct, ocs],
                                start=(ct == 0),
                                stop=False,
                            )
                        nc.tensor.matmul(
                            o_ps[:], ones_sb[:1, :], bprime_sb[:, ocs],
                            start=False, stop=True,
                        )
                        o_sb = workp.tile([128, 512], F32, tag="osb")
                        nc.vector.tensor_copy(out=o_sb[:], in_=o_ps[:])
                        nc.sync.dma_start(out_r[:, half * 4 + rt, ocs], o_sb[:])

            # ---------------- schedule ------------------------------------
            xts = {}
            for r in range(3):
                xts[r] = load_chunk(r)
            for r in range(16):
                if r + 3 < 16:
                    xts[r + 3] = load_chunk(r + 3)
                p1_chunk(r, xts.pop(r))
            for b in range(B):
                for h in range(HPC):
                    attn_chunk(b, h, 0)
                    attn_chunk(b, h, 2)
            flush_pending()
            fire_a2a(0)
            done = 0
            for b in range(B):
                for h in range(HPC):
                    attn_chunk(b, h, 1)
                    attn_chunk(b, h, 3)
                    done += 2
                    if done == 8:
                        # CC0 has completed by now; proj0's matmuls fill the
                        # PE slack of the ACT-paced attention region
                        proj_half(0, rts=(0, 1))
            flush_pending()
            fire_a2a(1)
            proj_half(0, rts=(2, 3))   # covers CC1's latency with PE work
            proj_half(1)

    nc.finalize()
    return nc


def _prep_inputs(x, c_attn_w, c_attn_b, c_proj_w, c_proj_b):
    x = np.asarray(x, dtype=np.float32)
    c_attn_w = np.asarray(c_attn_w, dtype=np.float32)
    c_attn_b = np.asarray(c_attn_b, dtype=np.float32)
    c_proj_w = np.asarray(c_proj_w, dtype=np.float32)
    c_proj_b = np.asarray(c_proj_b, dtype=np.float32)

    xT = np.ascontiguousarray(x.reshape(ROWS, C).T).astype(BF16)
    wq, wk, wv_full = c_attn_w[:, :C], c_attn_w[:, C : 2 * C], c_attn_w[:, 2 * C :]
    bqf, bkf, bvf = c_attn_b[:C], c_attn_b[C : 2 * C], c_attn_b[2 * C :]
    wp_b = np.ascontiguousarray(c_proj_w).astype(BF16)
    bprime = (bvf @ c_proj_w + c_proj_b).reshape(1, C).astype(BF16)
    mask = np.triu(np.ones((128, 128), dtype=np.float32)).astype(BF16)

    in_maps = []
    for c in range(NCORES):
        cs = slice(c * CPC, (c + 1) * CPC)
        in_maps.append(
            {
                "xT": xT,
                "wqk": np.ascontiguousarray(
                    np.concatenate([wq[:, cs], wk[:, cs]], axis=1)
                ).astype(BF16),
                "wv": np.ascontiguousarray(wv_full[:, cs]).astype(BF16),
                "bq": np.ascontiguousarray(bqf[cs].reshape(CPC, 1)).astype(np.float32),
                "bk": np.ascontiguousarray(bkf[cs].reshape(CPC, 1)).astype(np.float32),
                "wp": wp_b,
                "bprime": bprime,
                "maskd": mask,
            }
        )
    return in_maps


def kernel(x, c_attn_w, c_attn_b, c_proj_w, c_proj_b):
    from concourse.bass_utils import run_bass_kernel_spmd

    if "nc" not in _CACHE:
        _CACHE["nc"] = _build()
    nc = _CACHE["nc"]

    in_maps = _prep_inputs(x, c_attn_w, c_attn_b, c_proj_w, c_proj_b)
    res = run_bass_kernel_spmd(nc, in_maps, core_ids=list(range(NCORES)))
    full = np.concatenate([res.results[c]["out"] for c in range(NCORES)], axis=0)
    return full.reshape(B, T, C).astype(np.float32)
